# revision 1
# baseline (speedup 1.0000x reference)
"""VQ codebook squared-distance kernel for Trainium2 (8 NeuronCores).

Computes dist[n,k,l] = (||x[n,:,l]||^2 + ||w[k,:]||^2 - 2*x[n,:,l].w[k,:]) / scale^2
for x (32,128,3136) f32, weight (64,128) f32, scale (1,) f32 -> out (32,64,3136) f32.

Sharding: data-parallel over N (4 per core); weight/scale replicated.
The kernel is HBM-bound: 9.64 MB/core over a stack shared with the paired
core caps at ~310 GB/s, so the structure exists to keep the DMA stream
saturated; all compute hides under it.

Per-core design (fp16 PE path):
  - inputs: 8 fp32 half-tiles, all on the sync HWDGE ring (clean trigger
    FIFO); outputs on the scalar ring (disjoint trigger FIFO).
  - DVE casts x -> fp16 (2x_2P); ACT computes x^2 -> fp16 (Square, fp32 in).
  - PE: psum = (-2Wt)f16 @ x_f16 + ones_f16 @ (x^2)_f16, two n's per PSUM
    tile via column tiling (tile_position (0,0)/(0,64)); psum tiles span
    2 banks so one DVE epilogue covers 1024 cols:
    out = (psum + ||c_k||^2) / scale^2.
"""

import numpy as np

N, D, L, K = 32, 128, 3136, 64
N_CORES = 8
NS = N // N_CORES          # n's per core
LC = 392                   # matmul chunk (8 per row, one PSUM bank)
LH = L // 2                # half length for input DMA

_cache = {}


def _build():
    import concourse.bacc as bacc
    import concourse.mybir as mybir
    import concourse.tile as tile
    from concourse.masks import make_identity

    f32 = mybir.dt.float32
    f16 = mybir.dt.float16
    AF = mybir.ActivationFunctionType

    nc = bacc.Bacc(
        "TRN2",
        target_bir_lowering=False,
        debug=False,
        enable_asserts=False,
        num_devices=N_CORES,
    )

    x_ap = nc.dram_tensor("x", (NS, D, L), f32, kind="ExternalInput").ap()
    w_ap = nc.dram_tensor("weight", (K, D), f32, kind="ExternalInput").ap()
    s_ap = nc.dram_tensor("scale", (1,), f32, kind="ExternalInput").ap()
    o_ap = nc.dram_tensor("out", (NS, K, L), f32, kind="ExternalOutput").ap()

    with tile.TileContext(nc) as tc:
        with (
            tc.tile_pool(name="consts", bufs=1) as consts,
            tc.tile_pool(name="xin", bufs=4) as xpool,
            tc.tile_pool(name="xsq", bufs=4) as xqpool,
            tc.tile_pool(name="outp", bufs=2) as opool,
            tc.tile_pool(name="psum", bufs=4, space="PSUM") as pspool,
            tc.tile_pool(name="psum1", bufs=1, space="PSUM") as pspool1,
        ):
            # ---- input stream: SWDGE cast-on-load fp32->fp16 halves.
            # Pair-member interleaved order (n0h0, n1h0, n0h1, n1h1, ...):
            # the column-paired matmuls need BOTH images of a pair, so
            # loading both h0 halves first gates pair-0's first chunks on
            # transfer #2 instead of #3.
            stream = [(0, 0), (1, 0), (0, 1), (1, 1),
                      (2, 0), (3, 0), (2, 1), (3, 1)]
            xts = [
                xpool.tile([D, L], f16, tag="xt", name=f"x_{n}")
                for n in range(NS)
            ]
            for n, h in stream:
                hs = slice(h * LH, (h + 1) * LH)
                nc.gpsimd.dma_start(out=xts[n][:, hs], in_=x_ap[n][:, hs])

            # ---- constants -------------------------------------------------
            w2 = consts.tile([2 * K, D], f32)
            nc.sync.dma_start(out=w2[0:K, :], in_=w_ap)
            nc.sync.dma_start(out=w2[K : 2 * K, :], in_=w_ap)

            s_b = consts.tile([128, 1], f32)
            nc.gpsimd.dma_start(out=s_b, in_=s_ap.to_broadcast((128, 1)))
            inv_s2 = consts.tile([128, 1], f32)
            nc.vector.tensor_mul(inv_s2, s_b, s_b)
            nc.vector.reciprocal(inv_s2, inv_s2)

            ident = consts.tile([K, K], f32)
            make_identity(nc, ident)
            ps_w = pspool1.tile([D, K], f32)
            nc.tensor.transpose(ps_w, w2[0:K, :], ident)
            wT16 = consts.tile([D, K], f16)
            nc.vector.tensor_scalar_mul(wT16, in0=ps_w, scalar1=-2.0)

            ones16 = consts.tile([D, K], f16)
            nc.vector.memset(ones16, 1.0)

            w_sq = consts.tile([2 * K, D], f32)
            nc.vector.tensor_mul(w_sq, w2, w2)
            c_sq = consts.tile([2 * K, 1], f32)
            nc.vector.reduce_sum(out=c_sq, in_=w_sq, axis=mybir.AxisListType.X)

            # ---- PE warmup: dummy matmuls keep the HAM activity window busy
            # while inputs stream in, so real matmuls run at the warm clock
            # (measured 329 ns vs 534 ns cold per 392-col MM).
            warm_rhs = consts.tile([D, LC], f16)
            nc.vector.memset(warm_rhs, 0.0)
            warm_ps = pspool1.tile([D, LC], f32, name="warm_ps")
            for _ in range(24):
                nc.tensor.matmul(
                    warm_ps[0:K, :], wT16, warm_rhs,
                    start=True, stop=True, tile_position=(0, 0),
                )

            # ---- derived stream: fp16 x^2 per n (from fp16 x) -------------
            # squares emitted in arrival order (ACT is FIFO: matching the
            # stream order avoids head-of-line blocking)
            xqs = [
                xqpool.tile([D, L], f16, tag="xq", name=f"xsq_{n}")
                for n in range(NS)
            ]
            for n, h in stream:
                hs = slice(h * LH, (h + 1) * LH)
                nc.scalar.activation(xqs[n][:, hs], xts[n][:, hs], AF.Square)

            # ---- matmuls + epilogue per pair ------------------------------
            rings = [nc.sync, nc.scalar]
            ring_i = 0
            for pair in range(NS // 2):
                n0, n1 = 2 * pair, 2 * pair + 1
                out_t = opool.tile([2 * K, L], f32, tag="out_t", name=f"out_{pair}")
                for c in range(L // LC):
                    sl = slice(c * LC, (c + 1) * LC)
                    ps = pspool.tile([2 * K, LC], f32, name="ps")
                    nc.tensor.matmul(
                        ps[0:K, :], wT16, xts[n0][:, sl],
                        start=True, stop=False, tile_position=(0, 0),
                    )
                    nc.tensor.matmul(
                        ps[K : 2 * K, :], wT16, xts[n1][:, sl],
                        start=True, stop=False, tile_position=(0, 64),
                    )
                    nc.tensor.matmul(
                        ps[0:K, :], ones16, xqs[n0][:, sl],
                        start=False, stop=True, tile_position=(0, 0),
                    )
                    nc.tensor.matmul(
                        ps[K : 2 * K, :], ones16, xqs[n1][:, sl],
                        start=False, stop=True, tile_position=(0, 64),
                    )
                    nc.vector.tensor_scalar(
                        out=out_t[:, sl], in0=ps,
                        scalar1=c_sq, scalar2=inv_s2,
                        op0=mybir.AluOpType.add, op1=mybir.AluOpType.mult,
                    )
                o_pair = o_ap[2 * pair : 2 * pair + 2].rearrange("a k l -> (a k) l")
                if pair < NS // 2 - 1:
                    for h in range(2):
                        hs = slice(h * LH, (h + 1) * LH)
                        rings[ring_i % 2].dma_start(
                            out=o_pair[:, hs], in_=out_t[:, hs]
                        )
                        ring_i += 1
                else:
                    # taper the tail: last half ships as two concurrent
                    # quarters, one per HWDGE ring
                    hs = slice(0, LH)
                    rings[ring_i % 2].dma_start(out=o_pair[:, hs], in_=out_t[:, hs])
                    ring_i += 1
                    lq = L // 4
                    for q in (2, 3):
                        qs = slice(q * lq, (q + 1) * lq)
                        rings[q % 2].dma_start(out=o_pair[:, qs], in_=out_t[:, qs])

    nc.compile()
    return nc


def _get_nc():
    if "nc" not in _cache:
        _cache["nc"] = _build()
    return _cache["nc"]


def run(x, weight, scale, trace=False):
    from concourse.bass_utils import run_bass_kernel_spmd

    x = np.ascontiguousarray(np.asarray(x, dtype=np.float32))
    weight = np.ascontiguousarray(np.asarray(weight, dtype=np.float32))
    scale = np.ascontiguousarray(np.asarray(scale, dtype=np.float32))
    assert x.shape == (N, D, L) and weight.shape == (K, D) and scale.shape == (1,)

    nc = _get_nc()
    in_maps = [
        {"x": x[c * NS : (c + 1) * NS], "weight": weight, "scale": scale}
        for c in range(N_CORES)
    ]
    res = run_bass_kernel_spmd(
        nc, in_maps, core_ids=list(range(N_CORES)), trace=trace
    )
    out = np.concatenate([r["out"] for r in res.results], axis=0)
    return out, res


def kernel(x, weight, scale):
    out, _ = run(x, weight, scale, trace=False)
    return out



# revision 3
# speedup vs baseline: 1.1683x; 1.1683x over previous
"""VQ codebook squared-distance kernel for Trainium2 (8 NeuronCores).

Computes dist[n,k,l] = (||x[n,:,l]||^2 + ||w[k,:]||^2 - 2*x[n,:,l].w[k,:]) / scale^2
for x (32,128,3136) f32, weight (64,128) f32, scale (1,) f32 -> out (32,64,3136) f32.

Sharding: data-parallel over N (4 per core); weight/scale replicated.

Per-core design (v2 — streaming pipeline):
  - HBM traffic is the roofline: 6.42 MB x read (f32) + 1.61 MB out write
    (fp16, upcast to f32 on host; rel err ~2e-4 vs 2e-2 budget).
  - x loads via SWDGE cast-on-load f32->f16 in fine-grained transfers
    (quarters for the first pair, halves for the second) so the PE can
    start ~10us in and track the stream; output DMA (HWDGE, f16)
    overlaps the input stream instead of serializing after it.
  - PE: psum = (-2Wt)f16 @ x_f16 + ones_f16 @ (x^2)_f16, two n's per
    PSUM tile via column tiling (tile_position (0,0)/(0,64)).
  - squares on DVE (f16 2x); epilogue on ACT reading PSUM directly:
    out = Identity(psum * (1/s^2) + csq/s^2) -> f16 SBUF tile.
  - a few dummy-weight warmup matmuls raise the HAM activity clock
    before the real stream arrives.
"""

import numpy as np

N, D, L, K = 32, 128, 3136, 64
N_CORES = 8
NS = N // N_CORES          # n's per core
LC = 392                   # matmul chunk (8 per pair-row, one PSUM bank)
LQ = 784                   # input quarter for pair 0 (2 chunks)
LH = L // 2                # input half for pair 1

_cache = {}


def _build():
    import concourse.bacc as bacc
    import concourse.mybir as mybir
    import concourse.tile as tile
    from concourse.masks import make_identity

    f32 = mybir.dt.float32
    f16 = mybir.dt.float16
    AF = mybir.ActivationFunctionType

    nc = bacc.Bacc(
        "TRN2",
        target_bir_lowering=False,
        debug=False,
        enable_asserts=False,
        num_devices=N_CORES,
    )

    x_ap = nc.dram_tensor("x", (NS, D, L), f32, kind="ExternalInput").ap()
    w_ap = nc.dram_tensor("weight", (K, D), f32, kind="ExternalInput").ap()
    s_ap = nc.dram_tensor("scale", (1,), f32, kind="ExternalInput").ap()
    o_ap = nc.dram_tensor("out", (NS, K, L), f16, kind="ExternalOutput").ap()

    with tile.TileContext(nc) as tc:
        with (
            tc.tile_pool(name="consts", bufs=1) as consts,
            tc.tile_pool(name="xin", bufs=4) as xpool,
            tc.tile_pool(name="xsq", bufs=4) as xqpool,
            tc.tile_pool(name="outp", bufs=2) as opool,
            tc.tile_pool(name="psum", bufs=4, space="PSUM") as pspool,
            tc.tile_pool(name="psum1", bufs=1, space="PSUM") as pspool1,
        ):
            xts = [
                xpool.tile([D, L], f16, tag="xt", name=f"x_{n}")
                for n in range(NS)
            ]
            xqs = [
                xqpool.tile([D, L], f16, tag="xq", name=f"xsq_{n}")
                for n in range(NS)
            ]

            # ---- input stream (SWDGE Q0, cast f32->f16 on load).
            # gpsimd queue order is the descriptor-generation order: first
            # quarter first (PE lead-in), identity prep slotted right after
            # it, scale broadcast behind the first chunk-pair's data.
            stream = []  # (n, slice) in transfer order
            for q in range(4):
                qs = slice(q * LQ, (q + 1) * LQ)
                stream += [(0, qs), (1, qs)]
            for h in range(2):
                hs = slice(h * LH, (h + 1) * LH)
                stream += [(2, hs), (3, hs)]

            ident = consts.tile([K, K], f32)
            s_b = consts.tile([128, 1], f32)

            nc.gpsimd.dma_start(out=xts[0][:, stream[0][1]], in_=x_ap[0][:, stream[0][1]])
            make_identity(nc, ident)
            nc.gpsimd.dma_start(out=xts[1][:, stream[1][1]], in_=x_ap[1][:, stream[1][1]])
            nc.gpsimd.dma_start(out=s_b, in_=s_ap.to_broadcast((128, 1)))
            for n, sl in stream[2:]:
                nc.gpsimd.dma_start(out=xts[n][:, sl], in_=x_ap[n][:, sl])

            # ---- weight / scale prep (sync DMA + DVE + one tiny PE op) ----
            w2 = consts.tile([2 * K, D], f32)
            nc.sync.dma_start(out=w2[0:K, :], in_=w_ap)
            nc.sync.dma_start(out=w2[K : 2 * K, :], in_=w_ap)

            # PE warmup on zeroed dummy tiles (no dependency on real data):
            # keeps the HAM activity window busy so real matmuls run at the
            # warm clock.
            warm_w = consts.tile([D, K], f16)
            nc.vector.memset(warm_w, 0.0)
            warm_rhs = consts.tile([D, LC], f16)
            nc.vector.memset(warm_rhs, 0.0)
            warm_ps = pspool1.tile([D, LC], f32, name="warm_ps")
            for _ in range(8):
                nc.tensor.matmul(
                    warm_ps[0:K, :], warm_w, warm_rhs,
                    start=True, stop=True, tile_position=(0, 0),
                )

            ones16 = consts.tile([D, K], f16)
            nc.vector.memset(ones16, 1.0)

            w_sq = consts.tile([2 * K, D], f32)
            nc.vector.tensor_mul(w_sq, w2, w2)
            c_sq = consts.tile([2 * K, 1], f32)
            nc.vector.reduce_sum(out=c_sq, in_=w_sq, axis=mybir.AxisListType.X)

            ps_w = pspool1.tile([D, K], f32, name="ps_w")
            nc.tensor.transpose(ps_w, w2[0:K, :], ident)
            wT16 = consts.tile([D, K], f16)
            nc.vector.tensor_scalar_mul(wT16, in0=ps_w, scalar1=-2.0)

            inv_s2 = consts.tile([128, 1], f32)
            nc.vector.tensor_mul(inv_s2, s_b, s_b)
            nc.vector.reciprocal(inv_s2, inv_s2)
            c_sq_s = consts.tile([2 * K, 1], f32)
            nc.vector.tensor_mul(c_sq_s, c_sq, inv_s2)

            # ---- derived stream: fp16 x^2 on DVE, in arrival order -------
            for n, sl in stream:
                nc.vector.tensor_mul(xqs[n][:, sl], xts[n][:, sl], xts[n][:, sl])

            # ---- matmuls + ACT epilogue + output DMA per pair ------------
            rings = [nc.sync, nc.scalar]
            ring_i = 0
            for pair in range(NS // 2):
                n0, n1 = 2 * pair, 2 * pair + 1
                out_t = opool.tile([2 * K, L], f16, tag="out_t", name=f"out_{pair}")
                o_pair = o_ap[2 * pair : 2 * pair + 2].rearrange("a k l -> (a k) l")
                for c in range(L // LC):
                    sl = slice(c * LC, (c + 1) * LC)
                    ps = pspool.tile([2 * K, LC], f32, name="ps")
                    nc.tensor.matmul(
                        ps[0:K, :], wT16, xts[n0][:, sl],
                        start=True, stop=False, tile_position=(0, 0),
                    )
                    nc.tensor.matmul(
                        ps[K : 2 * K, :], wT16, xts[n1][:, sl],
                        start=True, stop=False, tile_position=(0, 64),
                    )
                    nc.tensor.matmul(
                        ps[0:K, :], ones16, xqs[n0][:, sl],
                        start=False, stop=True, tile_position=(0, 0),
                    )
                    nc.tensor.matmul(
                        ps[K : 2 * K, :], ones16, xqs[n1][:, sl],
                        start=False, stop=True, tile_position=(0, 64),
                    )
                    nc.scalar.activation(
                        out_t[:, sl], ps, AF.Identity,
                        bias=c_sq_s, scale=inv_s2,
                    )
                    if c % 2 == 1:
                        qs = slice((c - 1) * LC, (c + 1) * LC)
                        rings[ring_i % 2].dma_start(out=o_pair[:, qs], in_=out_t[:, qs])
                        ring_i += 1

    nc.compile()
    return nc


def _get_nc():
    if "nc" not in _cache:
        _cache["nc"] = _build()
    return _cache["nc"]


def run(x, weight, scale, trace=False):
    from concourse.bass_utils import run_bass_kernel_spmd

    x = np.ascontiguousarray(np.asarray(x, dtype=np.float32))
    weight = np.ascontiguousarray(np.asarray(weight, dtype=np.float32))
    scale = np.ascontiguousarray(np.asarray(scale, dtype=np.float32))
    assert x.shape == (N, D, L) and weight.shape == (K, D) and scale.shape == (1,)

    nc = _get_nc()
    in_maps = [
        {"x": x[c * NS : (c + 1) * NS], "weight": weight, "scale": scale}
        for c in range(N_CORES)
    ]
    res = run_bass_kernel_spmd(
        nc, in_maps, core_ids=list(range(N_CORES)), trace=trace
    )
    out = np.concatenate([r["out"] for r in res.results], axis=0).astype(np.float32)
    return out, res


def kernel(x, weight, scale):
    out, _ = run(x, weight, scale, trace=False)
    return out


# revision 5
# speedup vs baseline: 1.2137x; 1.0388x over previous
"""VQ codebook squared-distance kernel for Trainium2 (8 NeuronCores).

Computes dist[n,k,l] = (||x[n,:,l]||^2 + ||w[k,:]||^2 - 2*x[n,:,l].w[k,:]) / scale^2
for x (32,128,3136) f32, weight (64,128) f32, scale (1,) f32 -> out (32,64,3136) f32.

Sharding: data-parallel over N (4 per core); weight/scale replicated.

Per-core design (v3 — streaming pipeline):
  - HBM traffic is the roofline: 6.42 MB x read (f32) + 1.61 MB out write
    (fp16, upcast to f32 on host; rel err ~2e-4 vs 2e-2 budget).
  - x loads via SWDGE cast-on-load f32->f16; transfer granularity is
    eighths (one matmul chunk) at the stream head and tail, quarters in
    between, so the PE starts ~10us in and the tail dependency chain
    after the last byte is one chunk deep. Output DMA (HWDGE, f16)
    overlaps the input stream.
  - scale is broadcast 1->128 partitions with a 1-col fp32 matmul
    (a broadcast DMA would stall the input ring for ~1.5us).
  - PE: psum = (-2Wt)f16 @ x_f16 + ones_f16 @ (x^2)_f16, two n's per
    PSUM tile via column tiling (tile_position (0,0)/(0,64)). No PE
    warmups: the HAM boost is a duty-cycle budget; throttled-clock
    matmuls track the DMA rate anyway, and banked boost credit covers
    catch-up.
  - squares on DVE (f16 2x); epilogue on ACT reading PSUM directly:
    out = Identity(psum * (1/s^2) + csq/s^2) -> f16 SBUF tile.
"""

import numpy as np

N, D, L, K = 32, 128, 3136, 64
N_CORES = 8
NS = N // N_CORES          # n's per core
LC = 392                   # matmul chunk (8 per pair-row, one PSUM bank)

_cache = {}


def _build():
    import concourse.bacc as bacc
    import concourse.mybir as mybir
    import concourse.tile as tile
    from concourse.masks import make_identity

    f32 = mybir.dt.float32
    f16 = mybir.dt.float16
    AF = mybir.ActivationFunctionType

    nc = bacc.Bacc(
        "TRN2",
        target_bir_lowering=False,
        debug=False,
        enable_asserts=False,
        num_devices=N_CORES,
    )

    x_ap = nc.dram_tensor("x", (NS, D, L), f32, kind="ExternalInput").ap()
    w_ap = nc.dram_tensor("weight", (K, D), f32, kind="ExternalInput").ap()
    s_ap = nc.dram_tensor("scale", (1,), f32, kind="ExternalInput").ap()
    o_ap = nc.dram_tensor("out", (NS, K, L), f16, kind="ExternalOutput").ap()

    def ch(a, b):  # cols covering chunks [a, b)
        return slice(a * LC, b * LC)

    # input transfer plan: (n, col-slice) in ring order = consumption order
    stream = [(0, ch(0, 1)), (1, ch(0, 1)), (0, ch(1, 2)), (1, ch(1, 2))]
    for q in range(1, 4):
        stream += [(0, ch(2 * q, 2 * q + 2)), (1, ch(2 * q, 2 * q + 2))]
    for q in range(3):
        stream += [(2, ch(2 * q, 2 * q + 2)), (3, ch(2 * q, 2 * q + 2))]
    stream += [(2, ch(6, 7)), (3, ch(6, 7)), (2, ch(7, 8)), (3, ch(7, 8))]

    with tile.TileContext(nc) as tc:
        with (
            tc.tile_pool(name="consts", bufs=1) as consts,
            tc.tile_pool(name="xin", bufs=4) as xpool,
            tc.tile_pool(name="xsq", bufs=4) as xqpool,
            tc.tile_pool(name="outp", bufs=2) as opool,
            tc.tile_pool(name="psum", bufs=4, space="PSUM") as pspool,
            tc.tile_pool(name="psum1", bufs=1, space="PSUM") as pspool1,
        ):
            xts = [
                xpool.tile([D, L], f16, tag="xt", name=f"x_{n}")
                for n in range(NS)
            ]
            xqs = [
                xqpool.tile([D, L], f16, tag="xq", name=f"xsq_{n}")
                for n in range(NS)
            ]

            # ---- input stream (SWDGE Q0, cast f32->f16 on load).
            # The first two transfers lead; identity prep rides behind them
            # on the gpsimd queue.
            ident = consts.tile([K, K], f32)
            for i, (n, sl) in enumerate(stream):
                nc.gpsimd.dma_start(out=xts[n][:, sl], in_=x_ap[n][:, sl])
                if i == 1:
                    make_identity(nc, ident)

            # ---- weight / scale prep ------------------------------------
            s_t = consts.tile([1, 1], f32)
            nc.sync.dma_start(out=s_t, in_=s_ap.to_broadcast((1, 1)))
            w2 = consts.tile([2 * K, D], f32)
            nc.sync.dma_start(out=w2[0:K, :], in_=w_ap)
            nc.sync.dma_start(out=w2[K : 2 * K, :], in_=w_ap)

            ones_row = consts.tile([1, 128], f32)
            nc.vector.memset(ones_row, 1.0)
            ones16 = consts.tile([D, K], f16)
            nc.vector.memset(ones16, 1.0)

            # broadcast scale to all 128 partitions via 1-col fp32 matmul
            ps_s = pspool1.tile([128, 1], f32, name="ps_s")
            nc.tensor.matmul(ps_s, ones_row, s_t, start=True, stop=True)
            s_b = consts.tile([128, 1], f32)
            nc.vector.tensor_scalar_mul(s_b, in0=ps_s, scalar1=1.0)
            inv_s2 = consts.tile([128, 1], f32)
            nc.vector.tensor_mul(inv_s2, s_b, s_b)
            nc.vector.reciprocal(inv_s2, inv_s2)

            w_sq = consts.tile([2 * K, D], f32)
            nc.vector.tensor_mul(w_sq, w2, w2)
            c_sq = consts.tile([2 * K, 1], f32)
            nc.vector.reduce_sum(out=c_sq, in_=w_sq, axis=mybir.AxisListType.X)
            c_sq_s = consts.tile([2 * K, 1], f32)
            nc.vector.tensor_mul(c_sq_s, c_sq, inv_s2)

            ps_w = pspool1.tile([D, K], f32, name="ps_w")
            nc.tensor.transpose(ps_w, w2[0:K, :], ident)
            wT16 = consts.tile([D, K], f16)
            nc.vector.tensor_scalar_mul(wT16, in0=ps_w, scalar1=-2.0)

            # ---- derived stream: fp16 x^2 on DVE, in arrival order -------
            for n, sl in stream:
                nc.vector.tensor_mul(xqs[n][:, sl], xts[n][:, sl], xts[n][:, sl])

            # ---- matmuls + ACT epilogue + output DMA per pair ------------
            rings = [nc.sync, nc.scalar]
            ring_i = 0
            for pair in range(NS // 2):
                n0, n1 = 2 * pair, 2 * pair + 1
                out_t = opool.tile([2 * K, L], f16, tag="out_t", name=f"out_{pair}")
                o_pair = o_ap[2 * pair : 2 * pair + 2].rearrange("a k l -> (a k) l")
                for c in range(L // LC):
                    sl = ch(c, c + 1)
                    ps = pspool.tile([2 * K, LC], f32, name="ps")
                    nc.tensor.matmul(
                        ps[0:K, :], wT16, xts[n0][:, sl],
                        start=True, stop=False, tile_position=(0, 0),
                    )
                    nc.tensor.matmul(
                        ps[K : 2 * K, :], wT16, xts[n1][:, sl],
                        start=True, stop=False, tile_position=(0, 64),
                    )
                    nc.tensor.matmul(
                        ps[0:K, :], ones16, xqs[n0][:, sl],
                        start=False, stop=True, tile_position=(0, 0),
                    )
                    nc.tensor.matmul(
                        ps[K : 2 * K, :], ones16, xqs[n1][:, sl],
                        start=False, stop=True, tile_position=(0, 64),
                    )
                    nc.scalar.activation(
                        out_t[:, sl], ps, AF.Identity,
                        bias=c_sq_s, scale=inv_s2,
                    )
                    # ship finished columns: quarters, except the final
                    # quarter of the last pair which goes per-chunk to
                    # shorten the tail dependency chain
                    last_pair = pair == NS // 2 - 1
                    if last_pair and c >= 6:
                        rings[ring_i % 2].dma_start(
                            out=o_pair[:, sl], in_=out_t[:, sl]
                        )
                        ring_i += 1
                    elif c % 2 == 1:
                        qs = ch(c - 1, c + 1)
                        rings[ring_i % 2].dma_start(out=o_pair[:, qs], in_=out_t[:, qs])
                        ring_i += 1

    nc.compile()
    return nc


def _get_nc():
    if "nc" not in _cache:
        _cache["nc"] = _build()
    return _cache["nc"]


def run(x, weight, scale, trace=False):
    from concourse.bass_utils import run_bass_kernel_spmd

    x = np.ascontiguousarray(np.asarray(x, dtype=np.float32))
    weight = np.ascontiguousarray(np.asarray(weight, dtype=np.float32))
    scale = np.ascontiguousarray(np.asarray(scale, dtype=np.float32))
    assert x.shape == (N, D, L) and weight.shape == (K, D) and scale.shape == (1,)

    nc = _get_nc()
    in_maps = [
        {"x": x[c * NS : (c + 1) * NS], "weight": weight, "scale": scale}
        for c in range(N_CORES)
    ]
    res = run_bass_kernel_spmd(
        nc, in_maps, core_ids=list(range(N_CORES)), trace=trace
    )
    out = np.concatenate([r["out"] for r in res.results], axis=0).astype(np.float32)
    return out, res


def kernel(x, weight, scale):
    out, _ = run(x, weight, scale, trace=False)
    return out
